# revision 12
# baseline (speedup 1.0000x reference)
"""Trainium2 Bass kernel for nn_GaussianSplattingDecoder.

Splat 2048 gaussians onto a 200x200x16 voxel grid (V=640000), then a tiny
per-voxel MLP.  Exploits the radius-3 interaction mask: gaussian means are
~N(0,1) while the grid spans +-40 in x/y, so only ~3% of voxel tiles
interact with any gaussian at all.

Strategy (8 NeuronCores, SPMD — one program, per-core data):
  - Voxel tiles of TW=160 contiguous voxels.  Host finds, per tile, the
    candidate gaussians (dist(mean, tile bbox) < 3), packs them into blocks
    of 128 with tile-centered quadratic-form coefficients so both
      A = 0.5*mahalanobis - ln(opacity)   and   B = squared distance
    are K=8 matmuls (features [x'^2 y'^2 z'^2 x' y' z' 1 0]).
  - Device, per (tile, block) unit:  w = exp(-A) * (B < 9);  then
    psum2[18, TW] += semT.T @ w  (semantics cols 0..16, col 17 = 1 -> ws).
  - Per-tile epilogue: r = 1/max(ws, 1e-6), occ = psum2[:17]*r (PE
    broadcast of r), MLP (relu(W1@occ+b1), W2@h+b2), PE transpose, DMA out.
  - Inactive voxels: output is the constant c0 = W2@relu(b1)+b2; each core
    writes a c0-filled (V/8, 17) buffer; active tiles are computed into
    slot-indexed buffers and scattered over the fill on the host.
  - Active tiles are bucketed into block-count classes {1,2,4,8,16} and
    distributed round-robin so every core runs the identical static
    schedule (dummy all-zero slots pad the remainder; they are numerically
    inert and their outputs are ignored).
"""

import math
import numpy as np

import concourse.bass as bass
import concourse.bacc as bacc
import concourse.mybir as mybir
from concourse import tile
from concourse.bass_utils import run_bass_kernel_spmd

AF = mybir.ActivationFunctionType
ALU = mybir.AluOpType
F32 = mybir.dt.float32

OCC = (200, 200, 16)
V = OCC[0] * OCC[1] * OCC[2]
C = 17
R2 = 9.0
TW = 160           # voxels per tile
BLK = 128          # gaussians per block
N_CORES = 8
CLASSES = (1, 2, 4, 8, 16)
VPC = V // N_CORES  # voxels per core (fill slab)


# ----------------------------------------------------------------- host math
def _softplus64(x):
    return np.logaddexp(0.0, x.astype(np.float64))


def _log_sigmoid64(x):
    x = x.astype(np.float64)
    return np.where(x >= 0, -np.log1p(np.exp(-np.abs(x))),
                    x - np.log1p(np.exp(-np.abs(x))))


def _plan_and_pack(gaussian_props, voxel_coords):
    """Compute the sparse schedule and per-core packed inputs."""
    gp = np.asarray(gaussian_props, np.float32)[0]          # (N, 28)
    vc = np.asarray(voxel_coords, np.float32)               # (V, 3)
    means = gp[:, :3]
    scales = _softplus64(gp[:, 3:6]).astype(np.float32)
    inv_s = (1.0 / np.clip(scales * scales, 1e-6, None)).astype(np.float32)
    logop = _log_sigmoid64(gp[:, 10]).astype(np.float32)
    sem = gp[:, 11:11 + C]

    nt = V // TW
    vt = vc.reshape(nt, TW, 3)
    lo, hi = vt.min(1), vt.max(1)

    # candidate gaussians per tile: dist(mean, bbox) < 3
    tiles = []  # (tile_id, idx array)
    for s in range(0, nt, 1024):
        e = min(s + 1024, nt)
        cl = np.clip(means[None, :, :], lo[s:e, None, :], hi[s:e, None, :])
        d2 = ((cl - means[None, :, :]) ** 2).sum(-1)
        for i in range(e - s):
            idx = np.nonzero(d2[i] < R2)[0]
            if len(idx):
                tiles.append((s + i, idx))

    # bucket into classes, round-robin across cores
    by_class = {J: [] for J in CLASSES}
    for tid, idx in tiles:
        nb = (len(idx) + BLK - 1) // BLK
        J = next(c for c in CLASSES if c >= nb)
        by_class[J].append((tid, idx))
    counts = {J: (len(by_class[J]) + N_CORES - 1) // N_CORES for J in CLASSES}
    schedule = [(J, counts[J]) for J in CLASSES if counts[J] > 0]
    S = sum(cnt for _, cnt in schedule)          # slots per core
    U = sum(J * cnt for J, cnt in schedule)      # units per core

    feats = np.zeros((N_CORES, S, 8, TW), np.float32)
    lhs = np.zeros((N_CORES, U, 8, 2 * BLK), np.float32)
    semt = np.zeros((N_CORES, U, BLK, C + 1), np.float32)
    # (core, slot) -> tile_id for output scatter; -1 = dummy
    slot_tile = np.full((N_CORES, S), -1, np.int64)

    for core in range(N_CORES):
        sid = 0
        uid = 0
        for J, cnt in schedule:
            mine = by_class[J][core::N_CORES]
            for s in range(cnt):
                if s < len(mine):
                    tid, idx = mine[s]
                    slot_tile[core, sid] = tid
                    ctr = 0.5 * (lo[tid] + hi[tid])
                    x = vt[tid] - ctr[None, :]
                    feats[core, sid, 0:3] = (x * x).T
                    feats[core, sid, 3:6] = x.T
                    feats[core, sid, 6] = 1.0
                    m = means[idx] - ctr[None, :]
                    iv = inv_s[idx]
                    n = len(idx)
                    cA = np.zeros((8, J * BLK), np.float32)
                    cS = np.zeros((8, J * BLK), np.float32)
                    cA[0:3, :n] = (0.5 * iv).T
                    cA[3:6, :n] = (-iv * m).T
                    cA[6, :n] = 0.5 * (iv * m * m).sum(1) - logop[idx]
                    cA[6, n:] = 1e4     # padding: w = exp(-1e4) = 0
                    cS[0:3, :n] = 1.0
                    cS[3:6, :n] = (-2.0 * m).T
                    cS[6, :n] = (m * m).sum(1)
                    cS[6, n:] = 1e9     # padding: mask = 0
                    # col 0 = 1 (-> ws at psum partition 0, engine reads
                    # must start at partition 0/32/64/96), cols 1.. = sem
                    sT = np.zeros((J * BLK, C + 1), np.float32)
                    sT[:n, 0] = 1.0
                    sT[:n, 1:] = sem[idx]
                    for j in range(J):
                        lhs[core, uid + j, :, 0:BLK] = cA[:, j*BLK:(j+1)*BLK]
                        lhs[core, uid + j, :, BLK:] = cS[:, j*BLK:(j+1)*BLK]
                        semt[core, uid + j] = sT[j*BLK:(j+1)*BLK]
                # dummy slots stay all-zero (w=1 but sem=ws=0 -> out=c0)
                sid += 1
                uid += J
    return {
        "schedule": schedule, "S": S, "U": U, "slot_tile": slot_tile,
        "feats": feats, "lhs": lhs, "semt": semt,
    }


# ------------------------------------------------------------- bass program
def _build_program(schedule, S, U):
    nc = bacc.Bacc("TRN2", target_bir_lowering=False, debug=False,
                   num_devices=N_CORES)

    def din(name, shape):
        return nc.dram_tensor(name, list(shape), F32, kind="ExternalInput").ap()

    def dout(name, shape):
        return nc.dram_tensor(name, list(shape), F32, kind="ExternalOutput").ap()

    feats_d = din("feats", (S, 8, TW))
    lhs_d = din("lhs", (U, 8, 2 * BLK))
    semt_d = din("semt", (U, BLK, C + 1))
    w1t_d = din("w1t", (C + 1, 2 * C))  # row 0 zero (ignores ws row of occ)
    b1_d = din("b1", (2 * C, 1))
    w2t_d = din("w2t", (2 * C, C))
    b2_d = din("b2", (C, 1))
    b2row_d = din("b2row", (1, C))
    eye_d = din("eye", (C, C))
    fill_d = dout("fill", (VPC, C))
    slots_d = dout("slots", (S, TW, C))

    FILL_F = VPC * C // 128           # fill free-dim per partition (10625)
    FILL_CH = 5                       # fill DMA chunks
    assert FILL_F % (C * FILL_CH) == 0

    with tile.TileContext(nc) as tc:
        with (
            tc.tile_pool(name="const", bufs=1) as constp,
            tc.tile_pool(name="fillp", bufs=1) as fillp,
            tc.tile_pool(name="featp", bufs=2) as featp,
            tc.tile_pool(name="lhsp", bufs=4) as lhsp,
            tc.tile_pool(name="semp", bufs=4) as semp,
            tc.tile_pool(name="wp", bufs=4) as wp,
            tc.tile_pool(name="ep", bufs=3) as ep,
            tc.tile_pool(name="psab", bufs=4, space="PSUM") as psab,
            tc.tile_pool(name="ps2", bufs=2, space="PSUM") as ps2p,
            tc.tile_pool(name="pse", bufs=2, space="PSUM") as psep,
        ):
            # constants
            w1t_s = constp.tile([C + 1, 2 * C], F32, tag="w1t")
            nc.sync.dma_start(w1t_s[:], w1t_d[:])
            b1_s = constp.tile([2 * C, 1], F32, tag="b1")
            nc.sync.dma_start(b1_s[:], b1_d[:])
            w2t_s = constp.tile([2 * C, C], F32, tag="w2t")
            nc.sync.dma_start(w2t_s[:], w2t_d[:])
            b2_s = constp.tile([C, 1], F32, tag="b2")
            nc.sync.dma_start(b2_s[:], b2_d[:])
            b2row_s = constp.tile([1, C], F32, tag="b2row")
            nc.sync.dma_start(b2row_s[:], b2row_d[:])
            eye_s = constp.tile([C, C], F32, tag="eye")
            nc.sync.dma_start(eye_s[:], eye_d[:])
            ones_s = constp.tile([1, 128], F32, tag="ones")
            nc.vector.memset(ones_s[:], 1.0)

            # c0 = W2 @ relu(b1) + b2, as a row vector
            h0_s = constp.tile([2 * C, 1], F32, tag="h0")
            nc.scalar.activation(h0_s[:], b1_s[:], AF.Relu)
            pc0 = psep.tile([1, C], F32, tag="pse")
            nc.tensor.matmul(pc0[:], h0_s[:], w2t_s[:], start=True, stop=True)
            c0row_s = constp.tile([1, C], F32, tag="c0row")
            nc.vector.tensor_tensor(c0row_s[:], pc0[:], b2row_s[:], op=ALU.add)

            # c0 fill of the whole per-core slab: broadcast c0 to all 128
            # partitions via PE, then replicate along the free dim
            pfill = psep.tile([128, C], F32, tag="pse")
            nc.tensor.matmul(pfill[:], ones_s[:, 0:128], c0row_s[:],
                             start=True, stop=True)
            f17_s = constp.tile([128, C], F32, tag="f17")
            nc.scalar.activation(f17_s[:], pfill[:], AF.Copy)
            fill_s = fillp.tile([128, FILL_F], F32, tag="fill")
            nc.vector.tensor_copy(
                fill_s[:].rearrange("p (k c) -> p k c", c=C),
                f17_s[:].unsqueeze(1).broadcast_to([128, FILL_F // C, C]),
            )
            fill_flat = fill_d.flatten().rearrange("(p f) -> p f", p=128)
            fchunk = FILL_F // FILL_CH
            for i in range(FILL_CH):
                nc.sync.dma_start(
                    fill_flat[:, i * fchunk:(i + 1) * fchunk],
                    fill_s[:, i * fchunk:(i + 1) * fchunk],
                )

            # main sparse loop
            sid = 0
            uid = 0
            for J, cnt in schedule:
                for _ in range(cnt):
                    feats_s = featp.tile([8, TW], F32, tag="feats")
                    nc.sync.dma_start(feats_s[:], feats_d[sid])
                    p2 = ps2p.tile([C + 1, TW], F32, tag="ps2")
                    for j in range(J):
                        u = uid + j
                        lhs_s = lhsp.tile([8, 2 * BLK], F32, tag="lhs")
                        nc.sync.dma_start(lhs_s[:], lhs_d[u])
                        semt_s = semp.tile([BLK, C + 1], F32, tag="semt")
                        nc.sync.dma_start(semt_s[:], semt_d[u])
                        pa = psab.tile([BLK, TW], F32, tag="psab")
                        pb = psab.tile([BLK, TW], F32, tag="psab")
                        nc.tensor.matmul(pa[:], lhs_s[:, 0:BLK], feats_s[:],
                                         start=True, stop=True)
                        nc.tensor.matmul(pb[:], lhs_s[:, BLK:], feats_s[:],
                                         start=True, stop=True)
                        we_s = wp.tile([BLK, TW], F32, tag="we")
                        nc.scalar.activation(we_s[:], pa[:], AF.Exp, scale=-1.0)
                        w_s = wp.tile([BLK, TW], F32, tag="w")
                        nc.vector.scalar_tensor_tensor(
                            w_s[:], pb[:], float(R2), we_s[:],
                            op0=ALU.is_lt, op1=ALU.mult)
                        nc.tensor.matmul(p2[:], semt_s[:], w_s[:],
                                         start=(j == 0), stop=(j == J - 1))
                    # epilogue: ws is p2 row 0; normalize all 18 rows (row 0
                    # becomes ~1, ignored via the zero first row of w1t)
                    r_s = ep.tile([1, TW], F32, tag="r")
                    nc.vector.tensor_scalar_max(r_s[:], p2[0:1, :], 1e-6)
                    nc.vector.reciprocal(r_s[:], r_s[:])
                    pr = psep.tile([C + 1, TW], F32, tag="pse")
                    nc.tensor.matmul(pr[:], ones_s[:, 0:C + 1], r_s[:],
                                     start=True, stop=True)
                    rb_s = ep.tile([C + 1, TW], F32, tag="rb")
                    nc.scalar.activation(rb_s[:], pr[:], AF.Copy)
                    occ_s = ep.tile([C + 1, TW], F32, tag="occ")
                    nc.vector.tensor_tensor(occ_s[:], p2[:], rb_s[:],
                                            op=ALU.mult)
                    ph = psep.tile([2 * C, TW], F32, tag="pse")
                    nc.tensor.matmul(ph[:], w1t_s[:], occ_s[:],
                                     start=True, stop=True)
                    h_s = ep.tile([2 * C, TW], F32, tag="h")
                    nc.scalar.activation(h_s[:], ph[:], AF.Relu, bias=b1_s[:])
                    po = psep.tile([C, TW], F32, tag="pse")
                    nc.tensor.matmul(po[:], w2t_s[:], h_s[:],
                                     start=True, stop=True)
                    o_s = ep.tile([C, TW], F32, tag="o")
                    nc.scalar.activation(o_s[:], po[:], AF.Identity,
                                         bias=b2_s[:])
                    for v0 in range(0, TW, 128):
                        vn = min(128, TW - v0)
                        pt = psep.tile([128, C], F32, tag="pse")
                        nc.tensor.transpose(pt[:vn, :], o_s[:, v0:v0 + vn],
                                            eye_s[:])
                        ot_s = ep.tile([128, C], F32, tag="ot")
                        nc.scalar.activation(ot_s[:vn, :], pt[:vn, :], AF.Copy)
                        nc.sync.dma_start(slots_d[sid, v0:v0 + vn, :],
                                          ot_s[:vn, :])
                    sid += 1
                    uid += J
    return nc


# ---------------------------------------------------------------- execution
def _execute(nc, plan, W1, b1, W2, b2, trace=False, **kw):
    w1t = np.zeros((C + 1, 2 * C), np.float32)
    w1t[1:] = W1.T
    consts = {
        "w1t": w1t,
        "b1": b1.reshape(2 * C, 1).astype(np.float32),
        "w2t": np.ascontiguousarray(W2.T).astype(np.float32),
        "b2": b2.reshape(C, 1).astype(np.float32),
        "b2row": b2.reshape(1, C).astype(np.float32),
        "eye": np.eye(C, dtype=np.float32),
    }
    in_maps = []
    for core in range(N_CORES):
        m = dict(consts)
        m["feats"] = plan["feats"][core]
        m["lhs"] = plan["lhs"][core]
        m["semt"] = plan["semt"][core]
        in_maps.append(m)
    if not nc.is_finalized():
        nc.finalize()
    return run_bass_kernel_spmd(nc, in_maps, list(range(N_CORES)),
                                trace=trace, **kw)


def _assemble(plan, results):
    out = np.empty((V, C), np.float32)
    for core in range(N_CORES):
        out[core * VPC:(core + 1) * VPC] = results[core]["fill"]
    slot_tile = plan["slot_tile"]
    for core in range(N_CORES):
        slots = results[core]["slots"]
        for sid in range(plan["S"]):
            tid = slot_tile[core, sid]
            if tid >= 0:
                out[tid * TW:(tid + 1) * TW] = slots[sid]
    return out.reshape(1, OCC[0], OCC[1], OCC[2], C)


def run(inputs, trace=False, **kw):
    """Full pipeline; returns (output, BassKernelResults)."""
    gp = np.asarray(inputs["gaussian_props"], np.float32)
    plan = _plan_and_pack(gp, inputs["voxel_coords"])
    nc = _build_program(plan["schedule"], plan["S"], plan["U"])
    res = _execute(nc, plan,
                   np.asarray(inputs["W1"], np.float32),
                   np.asarray(inputs["b1"], np.float32),
                   np.asarray(inputs["W2"], np.float32),
                   np.asarray(inputs["b2"], np.float32),
                   trace=trace, **kw)
    out = _assemble(plan, res.results)
    return out, res


def kernel(**inputs) -> np.ndarray:
    out, _ = run(inputs)
    return out
